# revision 56
# baseline (speedup 1.0000x reference)
"""Trainium2 Bass kernel for CHMSA (cross-covariance multi-head self-attention
with a ConvNorm qkv stem).

Problem (hardcoded):
  x         [16, 64, 64, 256] f32
  dw_kernel [3, 3, 1, 256]    depthwise 3x3, SAME
  bn_gamma/bn_beta [256]      per-channel affine after dwconv
  pw_kernel [256, 768]        1x1 conv -> qkv
  q_bias/v_bias [256]         qkv bias = concat([q_bias, 0, v_bias])
  scale     [8,1,1]           per-head logit scale, s = exp(min(scale, ln 100))
  proj_w    [256, 256], proj_b [256]

Sharding: pure data-parallel over batch: 16 images / 8 cores = 2 images/core.
No collectives.

Per-core dataflow (per image, N = 4096 tokens, C = 256):
  1. DMA x token-major [128,256] tiles; PE-transpose -> x^T channel-major.
  2. dwconv: 9 diagonal matmuls per PSUM tile (channel-major), gamma folded
     into the diagonal weights, beta added at eviction. SAME padding via
     ragged per-tap APs (center tap first with start=True).
  3. qkv: q,k token-major computed per token-chunk PAIR; per-head sums of
     squares via ACT square + GpSimd grouped reduce; w = rsqrt(sqq*sqk)
     (s folded into softmax instead); qs = q*w and k read DIRECTLY from
     PSUM into small bf16 tiles; v channel-major (bf16).
  4. attn gram accumulated per 4-head group as bf16 [128,128] matmuls over
     all 32 token chunks; softmax applies s_h during the ACT logit gather;
     32x32 DVE transposes -> attn^T (bf16).
  5. out_cm = attn^T-weighted v (channel-major, bf16), proj back to
     token-major, DMA out.
"""

import math

import numpy as np
import ml_dtypes

import concourse.bass as bass
import concourse.mybir as mybir
import concourse.tile as tile
from concourse import bacc
from concourse.bass_utils import run_bass_kernel_spmd

F32 = mybir.dt.float32
F32R = mybir.dt.float32r
BF16 = mybir.dt.bfloat16
AF = mybir.ActivationFunctionType
ALU = mybir.AluOpType

B, H, W, C = 16, 64, 64, 256
N = H * W              # 4096 tokens per image
HEADS = 8
HD = C // HEADS        # 32
NCORES = 8
IMGS = B // NCORES     # 2 images per core
NCH = C // 128         # 2 channel chunks
LOG_MAX_SCALE = float(np.log(100.0))

# dwconv tap offsets (dh, dw), center first so it can carry start=True with
# full-tile coverage; the ragged edge taps then accumulate.
TAPS = [(0, 0), (-1, -1), (-1, 0), (-1, 1), (0, -1), (0, 1), (1, -1), (1, 0), (1, 1)]

HBLK = 8               # h-rows per dwconv psum tile -> free dim 8*64 = 512
NBLK = N // 128        # 32 token chunks of 128

# ---- engine assignment knobs ----
VT_EVICT_DVE = True    # v eviction on DVE (tensor_scalar) instead of ACT
SQ_FOLD_POOL = True    # fold sq 512->256 on GpSimd before the DVE reduce
KCOPY_ACT = False      # k bf16 eviction on ACT instead of DVE
# dwconv taps moved off PE: list of (tap_index, "pool"|"dve") FMAs into yt.
# When nonempty, tap 0 becomes an ACT init op (carrying beta) and the PE
# psum partial is folded in with a DVE add instead of the ACT bias-eviction.
DW_OFFLOAD = []


def _r(ap):
    """View an fp32 AP as float32r for full-rate PE matmuls."""
    return ap if ap.dtype == F32R else ap.bitcast(F32R)


def _build_program(consts, add_qbias, add_pbias, reps=1):
    nc = bacc.Bacc()

    # x arrives channel-major ([IMGS, C, N], transposed host-side) so the
    # dwconv input tile loads directly with no PE transposes. Declared f32r
    # (byte-identical to f32) so the DMA-written tile can feed f32r matmuls
    # without a separate rounding pass.
    x_dr = nc.dram_tensor("x", [IMGS, C, N], F32R, kind="ExternalInput")
    out_dr = nc.dram_tensor("out", [IMGS, N, C], F32, kind="ExternalOutput")

    diag_dr = nc.inline_tensor(consts["diag"], "cdiag")        # [128, NCH, 9, 128]
    pwqk_dr = nc.inline_tensor(consts["pwqk"], "cpwqk")        # [128, NCH, 512]
    pwv_dr = nc.inline_tensor(consts["pwv"], "cpwv")           # [128, NCH, NCH, 128]
    projw_dr = nc.inline_tensor(consts["projw"], "cprojw")     # [128, NCH, 256] bf16
    beta_dr = nc.inline_tensor(consts["beta"], "cbeta")        # [128, NCH]
    vb_dr = nc.inline_tensor(consts["vb"], "cvb")              # [128, NCH]
    s_host = [float(v) for v in consts["s_host"]]              # python floats
    if add_qbias:
        qb_dr = nc.inline_tensor(consts["qb"], "cqb")          # [128, 256]
    if add_pbias:
        pb_dr = nc.inline_tensor(consts["pb"], "cpb")          # [128, 256]

    with tile.TileContext(nc) as tc:
        with (
            tc.tile_pool(name="singles", bufs=1) as singles,
            tc.tile_pool(name="xt", bufs=1) as xt_pool,
            tc.tile_pool(name="img_big", bufs=1) as img_pool,
            tc.tile_pool(name="sq", bufs=3) as sq_pool,
            tc.tile_pool(name="yv", bufs=3) as yv_pool,
            tc.tile_pool(name="wp", bufs=3) as wp_pool,
            tc.tile_pool(name="small", bufs=3) as small,
            tc.tile_pool(name="ostage", bufs=2) as ostage,
            tc.tile_pool(name="ps_mm", bufs=2, space="PSUM") as ps_mm,
            tc.tile_pool(name="ps_qk", bufs=3, space="PSUM") as ps_qk,
            tc.tile_pool(name="ps_c", bufs=2, space="PSUM") as ps_c,
            tc.tile_pool(name="ps_attn", bufs=1, space="PSUM") as ps_attn,
        ):
            # ---- constants into SBUF ----
            diag_sb = singles.tile([128, NCH, 9, 128], F32)
            nc.gpsimd.dma_start(diag_sb[:], diag_dr[:])
            pwqk_sb = singles.tile([128, NCH, 512], F32)
            nc.gpsimd.dma_start(pwqk_sb[:], pwqk_dr[:])
            pwv_sb = singles.tile([128, NCH, NCH, 128], F32)
            nc.gpsimd.dma_start(pwv_sb[:], pwv_dr[:])
            projw_sb = singles.tile([128, NCH, 256], BF16)
            nc.gpsimd.dma_start(projw_sb[:], projw_dr[:])
            beta_sb = singles.tile([128, NCH], F32)
            nc.gpsimd.dma_start(beta_sb[:], beta_dr[:])
            vb_sb = singles.tile([128, NCH], F32)
            nc.gpsimd.dma_start(vb_sb[:], vb_dr[:])
            if DW_OFFLOAD:
                kcol_dr = nc.inline_tensor(consts["kcol"], "ckcol")
                kcol_sb = singles.tile([128, NCH, 9], F32)
                nc.gpsimd.dma_start(kcol_sb[:], kcol_dr[:])
            if add_qbias:
                qb_sb = singles.tile([128, 256], F32)
                nc.gpsimd.dma_start(qb_sb[:], qb_dr[:])
            if add_pbias:
                pb_sb = singles.tile([128, 256], F32)
                nc.gpsimd.dma_start(pb_sb[:], pb_dr[:])

            def make_img_state(img):
                st = {}
                st["img"] = img
                st["xt"] = xt_pool.tile([128, NCH, H + 2, W + 2], F32R,
                                        tag="xt", name=f"xt_{img}")
                xtf = st["xt"].bitcast(F32)
                nc.vector.memset(xtf[:, :, 0, :], 0.0)
                nc.vector.memset(xtf[:, :, H + 1, :], 0.0)
                nc.vector.memset(xtf[:, :, :, 0], 0.0)
                nc.vector.memset(xtf[:, :, :, W + 1], 0.0)
                st["yt"] = img_pool.tile([128, NCH, N], F32, tag="yt",
                                         name=f"yt{img}")
                st["vt"] = img_pool.tile([128, NCH, N], BF16, tag="vt",
                                         name=f"vt{img}")
                # one PSUM bank; head-group 0 accumulates pairwise during
                # phase A, group 1 re-runs over the stored qs/kb at the end
                # (start_tensor_calc zeroes the whole 2KB bank, so two
                # concurrently-open groups cannot share one).
                st["att"] = ps_attn.tile([128, 128], F32, tag="att",
                                         name=f"att_{img}")
                st["qs_all"] = img_pool.tile([128, NBLK, 256], BF16, tag="qsa",
                                             name=f"qsa{img}")
                st["kb_all"] = img_pool.tile([128, NBLK, 256], BF16, tag="kba",
                                             name=f"kba{img}")
                return st

            def load_rb(st, rb):
                # DMA one 8-row block (512 tokens) of channel-major x into
                # the padded xt tile, both channel chunks.
                img = st["img"]
                for cch in range(NCH):
                    nc.sync.dma_start(
                        st["xt"][:, cch, 1 + 8 * rb:9 + 8 * rb, 1:1 + W],
                        x_dr[img, cch * 128:(cch + 1) * 128,
                             rb * 512:(rb + 1) * 512].rearrange(
                                 "p (h w) -> p h w", w=W),
                    )

            def _xw(st, cch, hb, ti):
                # shifted dwconv input window for tap ti over hb's rows
                dh, dw = TAPS[ti]
                h0 = hb * HBLK
                return st["xt"][:, cch, 1 + h0 + dh:1 + h0 + HBLK + dh,
                                1 + dw:1 + W + dw]

            def dwconv_block(st, hb):
                h0 = hb * HBLK
                off = {ti: eng for ti, eng in DW_OFFLOAD}
                pe_taps = [ti for ti in range(9) if ti != 0 and ti not in off]
                for cch in range(NCH):
                    ysl = st["yt"][:, cch, h0 * W:(h0 + HBLK) * W]
                    ysl3 = ysl.rearrange("p (h w) -> p h w", w=W)
                    if off:
                        # center tap + beta as ACT init and offloaded-tap
                        # FMAs accumulate in a f32 scratch tile; the final
                        # DVE fold adds the PE psum partial and writes the
                        # f32r-rounded yt the qk/v matmuls require.
                        yv = yv_pool.tile([128, HBLK, W], F32, tag="yv",
                                          name="yv")
                        nc.scalar.activation(
                            out=yv[:], in_=_xw(st, cch, hb, 0).bitcast(F32),
                            func=AF.Identity,
                            scale=kcol_sb[:, cch, 0:1],
                            bias=beta_sb[:, cch:cch + 1],
                        )
                        for ti, eng in DW_OFFLOAD:
                            e = nc.gpsimd if eng == "pool" else nc.vector
                            e.scalar_tensor_tensor(
                                out=yv[:], in0=_xw(st, cch, hb, ti).bitcast(F32),
                                scalar=kcol_sb[:, cch, ti:ti + 1],
                                in1=yv[:], op0=ALU.mult, op1=ALU.add,
                            )
                    yp = ps_mm.tile([128, HBLK * W], F32, tag="mm", name="yp")
                    for i, ti in enumerate(pe_taps if off else range(9)):
                        nc.tensor.matmul(
                            yp[:],
                            _r(diag_sb[:, cch, ti, :]),
                            _r(_xw(st, cch, hb, ti)),
                            start=(i == 0),
                            stop=(i == (len(pe_taps) if off else 9) - 1),
                            skip_group_check=True,
                        )
                    if off:
                        nc.vector.scalar_tensor_tensor(
                            out=_r(ysl), in0=yp[:],
                            scalar=1.0,
                            in1=yv.rearrange("p h w -> p (h w)"),
                            op0=ALU.mult, op1=ALU.add)
                    else:
                        nc.scalar.activation(
                            out=_r(ysl),
                            in_=yp[:],
                            func=AF.Identity,
                            bias=beta_sb[:, cch:cch + 1],
                        )

            def v_block(st, nb):
                for vc in range(NCH):
                    vp = ps_mm.tile([128, 512], F32, tag="mm", name="vp")
                    for kc in range(NCH):
                        nc.tensor.matmul(
                            vp[:],
                            _r(pwv_sb[:, kc, vc, :]),
                            _r(st["yt"][:, kc, nb * 512:(nb + 1) * 512]),
                            start=(kc == 0),
                            stop=(kc == NCH - 1),
                        )
                    if VT_EVICT_DVE:
                        nc.vector.tensor_scalar(
                            out=st["vt"][:, vc, nb * 512:(nb + 1) * 512],
                            in0=vp[:], scalar1=vb_sb[:, vc:vc + 1],
                            scalar2=None, op0=ALU.add,
                        )
                    else:
                        nc.scalar.activation(
                            out=st["vt"][:, vc, nb * 512:(nb + 1) * 512],
                            in_=vp[:], func=AF.Identity,
                            bias=vb_sb[:, vc:vc + 1],
                        )

            def qk_front(st, p):
                # two token chunks t0,t1: qkv matmuls -> squares (ACT) ->
                # grouped reduce (GpSimd) -> w = rsqrt(sqq*sqk) -> qs (bf16,
                # q*w straight from PSUM) and k (bf16) evictions. The gram
                # matmuls are emitted later (qk_gram) so other PE work covers
                # this vector-side latency.
                qps = []
                sqs = []
                for j in (0, 1):
                    t = 2 * p + j
                    qp = ps_qk.tile([128, 512], F32, tag="qk", name=f"qp{j}")
                    for kc in range(NCH):
                        nc.tensor.matmul(
                            qp[:],
                            _r(st["yt"][:, kc, t * 128:(t + 1) * 128]),
                            _r(pwqk_sb[:, kc, :]),
                            start=(kc == 0),
                            stop=(kc == NCH - 1),
                        )
                    if add_qbias:
                        nc.vector.tensor_tensor(
                            out=qp[:, 0:256], in0=qp[:, 0:256],
                            in1=qb_sb[:], op=ALU.add,
                        )
                    sq = sq_pool.tile([128, 512], F32, name="sq")
                    nc.scalar.square(sq[:], qp[:])
                    qps.append(qp)
                    sqs.append(sq)
                sqr = wp_pool.tile([128, 2, 16], F32, tag="sqr", name="sqr")
                for j in (0, 1):
                    sqv = sqs[j].rearrange("p (g d) -> p g d", d=HD)
                    if SQ_FOLD_POOL:
                        sqh = sq_pool.tile([128, 16, 16], F32, tag="sqh",
                                           name="sqh")
                        nc.gpsimd.tensor_tensor(
                            out=sqh[:], in0=sqv[:, :, 0:16],
                            in1=sqv[:, :, 16:32], op=ALU.add,
                        )
                        sqv = sqh
                    nc.vector.tensor_reduce(
                        out=sqr[:, j, :],
                        in_=sqv,
                        axis=mybir.AxisListType.X,
                        op=ALU.add,
                    )
                # w = rsqrt(sqq*sqk) = sqrt(1/(sqq*sqk)); s folded into softmax
                w = wp_pool.tile([128, 2, 8], F32, tag="w", name="w")
                wf = w.rearrange("p a h -> p (a h)")
                nc.vector.tensor_tensor(
                    out=w[:], in0=sqr[:, :, 0:8], in1=sqr[:, :, 8:16],
                    op=ALU.mult,
                )
                nc.vector.reciprocal(wf, wf)
                nc.scalar.activation(wf, wf, AF.Sqrt)
                for j in (0, 1):
                    t = 2 * p + j
                    nc.vector.tensor_tensor(
                        out=st["qs_all"][:, t, :].rearrange(
                            "p (h d) -> p h d", d=HD),
                        in0=qps[j][:, 0:256].rearrange("p (h d) -> p h d", d=HD),
                        in1=w[:, j, :].unsqueeze(2).broadcast_to([128, 8, HD]),
                        op=ALU.mult,
                    )
                    if KCOPY_ACT:
                        nc.scalar.copy(st["kb_all"][:, t, :], qps[j][:, 256:512])
                    else:
                        nc.vector.tensor_copy(
                            st["kb_all"][:, t, :], qps[j][:, 256:512])

            def qk_gram(st, p, g=0):
                for j in (0, 1):
                    t = 2 * p + j
                    nc.tensor.matmul(
                        st["att"][:],
                        st["qs_all"][:, t, g * 128:(g + 1) * 128],
                        st["kb_all"][:, t, g * 128:(g + 1) * 128],
                        start=(t == 0),
                        stop=(t == NBLK - 1),
                    )

            def softmax_g(st, g):
                if g == 0:
                    st["at_bd"] = small.tile([128, 2, 128], BF16, tag="atbd",
                                             name="at_bd")
                at_bd = st["at_bd"]
                asm = small.tile([128, 32], F32, tag="asm", name="asm")
                for j in range(4):
                    h = 4 * g + j
                    nc.scalar.mul(
                        asm[32 * j:32 * j + 32, :],
                        st["att"][32 * j:32 * j + 32, 32 * j:32 * j + 32],
                        s_host[h],
                    )
                mx = small.tile([128, 1], F32, tag="mx", name="mx")
                nc.vector.tensor_reduce(
                    out=mx[:], in_=asm[:], axis=mybir.AxisListType.X,
                    op=ALU.max, negate=True)
                nc.scalar.activation(asm[:], asm[:], AF.Exp, bias=mx[:])
                sm = small.tile([128, 1], F32, tag="sm", name="sm")
                nc.vector.tensor_reduce(
                    out=sm[:], in_=asm[:], axis=mybir.AxisListType.X,
                    op=ALU.add)
                nc.vector.reciprocal(sm[:], sm[:])
                nc.vector.tensor_scalar(
                    out=asm[:], in0=asm[:], scalar1=sm[:], scalar2=None,
                    op0=ALU.mult)
                atf = small.tile([128, 128], F32, tag="atf", name="atf")
                nc.vector.memset(atf[:], 0.0)
                for j in range(4):
                    nc.vector.transpose(
                        atf[32 * j:32 * j + 32, 32 * j:32 * j + 32],
                        asm[32 * j:32 * j + 32, :],
                    )
                nc.vector.tensor_copy(at_bd[:, g, :], atf[:])

            def cv_block(st, nb, tail=False):
                # attn^T @ v for one 512-token slab (both head groups). In
                # tail mode the psum tiles borrow the idle ps_qk banks.
                img = st["img"]
                if nb == 0:
                    st["ocm"] = img_pool.tile([128, NCH, N], BF16, tag="ocm",
                                              name=f"ocm{img}")
                ocm = st["ocm"]
                for g in range(NCH):
                    op_ = (ps_qk if tail else ps_c).tile(
                        [128, 512], F32, tag="qk" if tail else "cmm",
                        name="op_")
                    nc.tensor.matmul(
                        op_[:],
                        st["at_bd"][:, g, :],
                        st["vt"][:, g, nb * 512:(nb + 1) * 512],
                    )
                    if g == 0:
                        nc.vector.tensor_copy(
                            ocm[:, g, nb * 512:(nb + 1) * 512], op_[:])
                    else:
                        nc.scalar.copy(
                            ocm[:, g, nb * 512:(nb + 1) * 512], op_[:])

            def proj_block(st, nb):
                img = st["img"]
                ocm = st["ocm"]
                for t in range(4 * nb, 4 * nb + 4):
                    pp = ps_c.tile([128, 256], F32, tag="cmm", name="pp")
                    for kc in range(NCH):
                        nc.tensor.matmul(
                            pp[:],
                            ocm[:, kc, t * 128:(t + 1) * 128],
                            projw_sb[:, kc, :],
                            start=(kc == 0),
                            stop=(kc == NCH - 1),
                        )
                    if t % 2 == 0:
                        ot = ostage.tile([128, 2, 256], F32, name="ot")
                        st["ot"] = ot
                    ot = st["ot"]
                    if add_pbias:
                        nc.vector.tensor_tensor(
                            out=ot[:, t % 2, :], in0=pp[:], in1=pb_sb[:],
                            op=ALU.add)
                    elif t % 2 == 0:
                        nc.scalar.copy(ot[:, t % 2, :], pp[:])
                    else:
                        nc.vector.tensor_copy(ot[:, t % 2, :], pp[:])
                    if t % 2 == 1:
                        nc.sync.dma_start(
                            out_dr[img, (t - 1) * 128:(t + 1) * 128,
                                   :].rearrange("(g p) c -> p g c", p=128),
                            ot[:],
                        )

            def c_block(st, nb, tail=False):
                cv_block(st, nb, tail)
                proj_block(st, nb)

            def phase_tail(st):
                # group-0 softmax, group-1 gram re-run, group-1 softmax.
                softmax_g(st, 0)
                for p in range(NBLK // 2):
                    qk_gram(st, p, g=1)
                softmax_g(st, 1)

            def phase_A(st, interleave=None, pre=None):
                # interleave: optional callable(tb) emitting prev-img C blocks
                def qk_v_body(hb):
                    # consumes yt of hb (evicted a full stage earlier, so PE
                    # never waits on the eviction). qk_gram(p) is deferred
                    # past qk_front(p+1) + a v_block so PE never stalls on
                    # the pair's vector-side chain either.
                    qk_front(st, 2 * hb)
                    qk_front(st, 2 * hb + 1)
                    if hb >= 2:
                        v_block(st, hb - 2)
                    qk_gram(st, 2 * hb)
                    qk_gram(st, 2 * hb + 1)

                for s in range(8):
                    if s == 0:
                        load_rb(st, 0)
                        load_rb(st, 1)
                    elif s < 7:
                        load_rb(st, s + 1)
                    if pre is not None and s == 0:
                        # previous image's tail: its g1 gram matmuls overlap
                        # this image's load DMAs, and its softmax vector
                        # chains overlap this image's first dwconv blocks.
                        # This image's own first gram (s=1) lands well after
                        # the att bank is read out.
                        pre()
                    if interleave is not None:
                        interleave(s)
                    dwconv_block(st, s)
                    if s >= 1:
                        qk_v_body(s - 1)
                qk_v_body(7)
                v_block(st, 6)
                v_block(st, 7)

            import contextlib
            rep_engines = (mybir.EngineType.PE, mybir.EngineType.DVE,
                           mybir.EngineType.Activation, mybir.EngineType.SP,
                           mybir.EngineType.Pool)
            rep_ctx = (tc.For_i(0, reps, 1, hint_engines=rep_engines)
                       if reps > 1 else contextlib.nullcontext())
            with rep_ctx:
                sts = []
                for img in range(IMGS):
                    st = make_img_state(img)
                    if not sts:
                        phase_A(st)
                    else:
                        pv = sts[-1]

                        def pre(pv=pv):
                            phase_tail(pv)

                        def emit_c(s, pv=pv):
                            # shifted by 2: slots 0-1 host pv's tail instead
                            if s >= 2:
                                c_block(pv, s - 2)
                        phase_A(st, interleave=emit_c, pre=pre)
                    sts.append(st)
                # leftover c blocks of the next-to-last image give the PE
                # dense work while the last image's tail vector chains run.
                last = sts[-1]
                if len(sts) > 1:
                    c_block(sts[-2], 6)
                    c_block(sts[-2], 7)
                phase_tail(last)
                # software-pipelined trailing: attn@v of slab nb+1 issues
                # before proj of slab nb so PE isn't gated on ocm evictions.
                cv_block(last, 0, tail=True)
                for nb in range(1, 8):
                    cv_block(last, nb, tail=True)
                    proj_block(last, nb - 1)
                proj_block(last, 7)

    nc.finalize()
    return nc


def _prep_consts(dw_kernel, bn_gamma, bn_beta, pw_kernel, q_bias, v_bias,
                 scale, proj_w, proj_b):
    taps_w = np.empty((9, C), np.float32)
    for ti, (dh, dw) in enumerate(TAPS):
        taps_w[ti] = dw_kernel[dh + 1, dw + 1, 0, :] * bn_gamma

    diag = np.zeros((128, NCH, 9, 128), np.float32)
    idx = np.arange(128)
    for cch in range(NCH):
        for ti in range(9):
            diag[idx, cch, ti, idx] = taps_w[ti, cch * 128 + idx]

    pwqk = np.empty((128, NCH, 512), np.float32)
    pwv = np.empty((128, NCH, NCH, 128), np.float32)
    for kc in range(NCH):
        pwqk[:, kc, :] = pw_kernel[kc * 128:(kc + 1) * 128, 0:512]
        for vc in range(NCH):
            pwv[:, kc, vc, :] = pw_kernel[kc * 128:(kc + 1) * 128,
                                          512 + vc * 128:512 + (vc + 1) * 128]

    projw = np.empty((128, NCH, 256), ml_dtypes.bfloat16)
    for kc in range(NCH):
        projw[:, kc, :] = proj_w[kc * 128:(kc + 1) * 128, :].astype(ml_dtypes.bfloat16)

    s_host = np.exp(np.minimum(scale.reshape(HEADS), LOG_MAX_SCALE)).astype(np.float32)

    # tap weights as per-partition columns [128, NCH, 9] for vector-engine
    # dwconv tap offload (scalar_tensor_tensor / activation scale operands)
    kcol = np.empty((128, NCH, 9), np.float32)
    for cch in range(NCH):
        for ti in range(9):
            kcol[:, cch, ti] = taps_w[ti, cch * 128:(cch + 1) * 128]

    consts = {
        "diag": diag,
        "pwqk": pwqk,
        "pwv": pwv,
        "projw": projw,
        "beta": bn_beta.reshape(NCH, 128).T.astype(np.float32).copy(),
        "vb": v_bias.reshape(NCH, 128).T.astype(np.float32).copy(),
        "kcol": kcol,
        "s_host": s_host,
        "qb": np.tile(q_bias[None, :], (128, 1)).astype(np.float32),
        "pb": np.tile(proj_b[None, :], (128, 1)).astype(np.float32),
    }
    return consts


def make_in_maps(x):
    # per-core channel-major x: [IMGS, C, N]
    xs = np.ascontiguousarray(
        np.asarray(x, np.float32).reshape(NCORES, IMGS, N, C).transpose(
            0, 1, 3, 2))
    return [{"x": xs[i]} for i in range(NCORES)]


def kernel(x, dw_kernel, bn_gamma, bn_beta, pw_kernel, q_bias, v_bias, scale,
           proj_w, proj_b):
    consts = _prep_consts(
        np.asarray(dw_kernel, np.float32), np.asarray(bn_gamma, np.float32),
        np.asarray(bn_beta, np.float32), np.asarray(pw_kernel, np.float32),
        np.asarray(q_bias, np.float32), np.asarray(v_bias, np.float32),
        np.asarray(scale, np.float32), np.asarray(proj_w, np.float32),
        np.asarray(proj_b, np.float32))

    add_qbias = bool(np.any(q_bias))
    add_pbias = bool(np.any(proj_b))
    nc = _build_program(consts, add_qbias, add_pbias)

    in_maps = make_in_maps(x)
    res = run_bass_kernel_spmd(nc, in_maps, core_ids=list(range(NCORES)))
    out = np.stack([res.results[i]["out"] for i in range(NCORES)])
    return out.reshape(B, H, W, C)


if __name__ == "__main__":
    pass


# revision 59
# speedup vs baseline: 1.0442x; 1.0442x over previous
"""Trainium2 Bass kernel for CHMSA (cross-covariance multi-head self-attention
with a ConvNorm qkv stem).

Problem (hardcoded):
  x         [16, 64, 64, 256] f32
  dw_kernel [3, 3, 1, 256]    depthwise 3x3, SAME
  bn_gamma/bn_beta [256]      per-channel affine after dwconv
  pw_kernel [256, 768]        1x1 conv -> qkv
  q_bias/v_bias [256]         qkv bias = concat([q_bias, 0, v_bias])
  scale     [8,1,1]           per-head logit scale, s = exp(min(scale, ln 100))
  proj_w    [256, 256], proj_b [256]

Sharding: pure data-parallel over batch: 16 images / 8 cores = 2 images/core.
No collectives.

Per-core dataflow (per image, N = 4096 tokens, C = 256):
  1. x is pre-transposed to channel-major [C, N] on the HOST (make_in_maps),
     so the dwconv input tile DMAs straight into SBUF with no PE transposes
     and no eviction pass (declared f32r, byte-identical to f32, to satisfy
     the fp32r-rounding dataflow check).
  2. dwconv: 9 diagonal f32r matmuls per PSUM tile (channel-major), gamma
     folded into the diagonal weights, beta added at the ACT eviction. SAME
     padding via the zeroed halo of the padded xt tile.
  3. qkv: q,k token-major computed per token-chunk PAIR: ACT square ->
     GpSimd half-fold -> DVE grouped reduce -> w = rsqrt(sqq*sqk) (the
     per-head scale s is folded into the softmax logit gather instead);
     qs = q*w and k are written DIRECTLY from PSUM into bf16 tiles;
     v channel-major (bf16).
  4. attn gram: bf16 [128,128] matmuls accumulated over all 32 token
     chunks. One PSUM bank only (start_tensor_calc zeroes a whole 2KB
     bank): head-group 0 streams pairwise behind the qk fronts, group 1
     re-runs from the stored qs/kb after group 0's softmax reads the bank.
     Softmax applies s_h during the ACT logit gather; 32x32 DVE transposes
     build attn^T (bf16).
  5. out_cm = attn^T-weighted v (channel-major, bf16), proj back to
     token-major (bf16 weights), DMA out. The previous image's C blocks
     interleave into the next image's phase to keep PE dense.
"""

import math

import numpy as np
import ml_dtypes

import concourse.bass as bass
import concourse.mybir as mybir
import concourse.tile as tile
from concourse import bacc
from concourse.bass_utils import run_bass_kernel_spmd

F32 = mybir.dt.float32
F32R = mybir.dt.float32r
BF16 = mybir.dt.bfloat16
AF = mybir.ActivationFunctionType
ALU = mybir.AluOpType

B, H, W, C = 16, 64, 64, 256
N = H * W              # 4096 tokens per image
HEADS = 8
HD = C // HEADS        # 32
NCORES = 8
IMGS = B // NCORES     # 2 images per core
NCH = C // 128         # 2 channel chunks
LOG_MAX_SCALE = float(np.log(100.0))

# dwconv tap offsets (dh, dw), center first so it can carry start=True with
# full-tile coverage; the ragged edge taps then accumulate.
TAPS = [(0, 0), (-1, -1), (-1, 0), (-1, 1), (0, -1), (0, 1), (1, -1), (1, 0), (1, 1)]

HBLK = 8               # h-rows per dwconv psum tile -> free dim 8*64 = 512
NBLK = N // 128        # 32 token chunks of 128

# ---- engine assignment knobs ----
VT_EVICT_DVE = False    # v eviction on DVE (tensor_scalar) instead of ACT
SQ_FOLD_POOL = True    # fold sq 512->256 on GpSimd before the DVE reduce
KCOPY_ACT = False      # k bf16 eviction on ACT instead of DVE
# dwconv taps moved off PE: list of (tap_index, "pool"|"dve") FMAs into yt.
# When nonempty, tap 0 becomes an ACT init op (carrying beta) and the PE
# psum partial is folded in with a DVE add instead of the ACT bias-eviction.
DW_OFFLOAD = []


def _r(ap):
    """View an fp32 AP as float32r for full-rate PE matmuls."""
    return ap if ap.dtype == F32R else ap.bitcast(F32R)


def _build_program(consts, add_qbias, add_pbias, reps=1):
    nc = bacc.Bacc()

    # x arrives channel-major ([IMGS, C, N], transposed host-side) so the
    # dwconv input tile loads directly with no PE transposes. Declared f32r
    # (byte-identical to f32) so the DMA-written tile can feed f32r matmuls
    # without a separate rounding pass.
    x_dr = nc.dram_tensor("x", [IMGS, C, N], F32R, kind="ExternalInput")
    out_dr = nc.dram_tensor("out", [IMGS, N, C], F32, kind="ExternalOutput")

    diag_dr = nc.inline_tensor(consts["diag"], "cdiag")        # [128, NCH, 9, 128]
    pwqk_dr = nc.inline_tensor(consts["pwqk"], "cpwqk")        # [128, NCH, 512]
    pwv_dr = nc.inline_tensor(consts["pwv"], "cpwv")           # [128, NCH, NCH, 128]
    projw_dr = nc.inline_tensor(consts["projw"], "cprojw")     # [128, NCH, 256] bf16
    beta_dr = nc.inline_tensor(consts["beta"], "cbeta")        # [128, NCH]
    vb_dr = nc.inline_tensor(consts["vb"], "cvb")              # [128, NCH]
    s_host = [float(v) for v in consts["s_host"]]              # python floats
    if add_qbias:
        qb_dr = nc.inline_tensor(consts["qb"], "cqb")          # [128, 256]
    if add_pbias:
        pb_dr = nc.inline_tensor(consts["pb"], "cpb")          # [128, 256]

    with tile.TileContext(nc) as tc:
        with (
            tc.tile_pool(name="singles", bufs=1) as singles,
            tc.tile_pool(name="xt", bufs=1) as xt_pool,
            tc.tile_pool(name="img_big", bufs=1) as img_pool,
            tc.tile_pool(name="sq", bufs=3) as sq_pool,
            tc.tile_pool(name="yv", bufs=3) as yv_pool,
            tc.tile_pool(name="wp", bufs=3) as wp_pool,
            tc.tile_pool(name="small", bufs=3) as small,
            tc.tile_pool(name="ostage", bufs=2) as ostage,
            tc.tile_pool(name="ps_mm", bufs=2, space="PSUM") as ps_mm,
            tc.tile_pool(name="ps_qk", bufs=3, space="PSUM") as ps_qk,
            tc.tile_pool(name="ps_c", bufs=2, space="PSUM") as ps_c,
            tc.tile_pool(name="ps_attn", bufs=1, space="PSUM") as ps_attn,
        ):
            # ---- constants into SBUF ----
            diag_sb = singles.tile([128, NCH, 9, 128], F32)
            nc.gpsimd.dma_start(diag_sb[:], diag_dr[:])
            pwqk_sb = singles.tile([128, NCH, 512], F32)
            nc.gpsimd.dma_start(pwqk_sb[:], pwqk_dr[:])
            pwv_sb = singles.tile([128, NCH, NCH, 128], F32)
            nc.gpsimd.dma_start(pwv_sb[:], pwv_dr[:])
            projw_sb = singles.tile([128, NCH, 256], BF16)
            nc.gpsimd.dma_start(projw_sb[:], projw_dr[:])
            beta_sb = singles.tile([128, NCH], F32)
            nc.gpsimd.dma_start(beta_sb[:], beta_dr[:])
            vb_sb = singles.tile([128, NCH], F32)
            nc.gpsimd.dma_start(vb_sb[:], vb_dr[:])
            if DW_OFFLOAD:
                kcol_dr = nc.inline_tensor(consts["kcol"], "ckcol")
                kcol_sb = singles.tile([128, NCH, 9], F32)
                nc.gpsimd.dma_start(kcol_sb[:], kcol_dr[:])
            if add_qbias:
                qb_sb = singles.tile([128, 256], F32)
                nc.gpsimd.dma_start(qb_sb[:], qb_dr[:])
            if add_pbias:
                pb_sb = singles.tile([128, 256], F32)
                nc.gpsimd.dma_start(pb_sb[:], pb_dr[:])

            def make_img_state(img):
                st = {}
                st["img"] = img
                st["xt"] = xt_pool.tile([128, NCH, H + 2, W + 2], F32R,
                                        tag="xt", name=f"xt_{img}")
                xtf = st["xt"].bitcast(F32)
                nc.vector.memset(xtf[:, :, 0, :], 0.0)
                nc.vector.memset(xtf[:, :, H + 1, :], 0.0)
                nc.vector.memset(xtf[:, :, :, 0], 0.0)
                nc.vector.memset(xtf[:, :, :, W + 1], 0.0)
                st["yt"] = img_pool.tile([128, NCH, N], F32, tag="yt",
                                         name=f"yt{img}")
                st["vt"] = img_pool.tile([128, NCH, N], BF16, tag="vt",
                                         name=f"vt{img}")
                # one PSUM bank; head-group 0 accumulates pairwise during
                # phase A, group 1 re-runs over the stored qs/kb at the end
                # (start_tensor_calc zeroes the whole 2KB bank, so two
                # concurrently-open groups cannot share one).
                st["att"] = ps_attn.tile([128, 128], F32, tag="att",
                                         name=f"att_{img}")
                st["qs_all"] = img_pool.tile([128, NBLK, 256], BF16, tag="qsa",
                                             name=f"qsa{img}")
                st["kb_all"] = img_pool.tile([128, NBLK, 256], BF16, tag="kba",
                                             name=f"kba{img}")
                return st

            def load_rb(st, rb):
                # DMA one 8-row block (512 tokens) of channel-major x into
                # the padded xt tile, both channel chunks.
                img = st["img"]
                for cch in range(NCH):
                    nc.sync.dma_start(
                        st["xt"][:, cch, 1 + 8 * rb:9 + 8 * rb, 1:1 + W],
                        x_dr[img, cch * 128:(cch + 1) * 128,
                             rb * 512:(rb + 1) * 512].rearrange(
                                 "p (h w) -> p h w", w=W),
                    )

            def _xw(st, cch, hb, ti):
                # shifted dwconv input window for tap ti over hb's rows
                dh, dw = TAPS[ti]
                h0 = hb * HBLK
                return st["xt"][:, cch, 1 + h0 + dh:1 + h0 + HBLK + dh,
                                1 + dw:1 + W + dw]

            def dwconv_block(st, hb):
                h0 = hb * HBLK
                off = {ti: eng for ti, eng in DW_OFFLOAD}
                pe_taps = [ti for ti in range(9) if ti != 0 and ti not in off]
                for cch in range(NCH):
                    ysl = st["yt"][:, cch, h0 * W:(h0 + HBLK) * W]
                    ysl3 = ysl.rearrange("p (h w) -> p h w", w=W)
                    if off:
                        # center tap + beta as ACT init and offloaded-tap
                        # FMAs accumulate in a f32 scratch tile; the final
                        # DVE fold adds the PE psum partial and writes the
                        # f32r-rounded yt the qk/v matmuls require.
                        yv = yv_pool.tile([128, HBLK, W], F32, tag="yv",
                                          name="yv")
                        nc.scalar.activation(
                            out=yv[:], in_=_xw(st, cch, hb, 0).bitcast(F32),
                            func=AF.Identity,
                            scale=kcol_sb[:, cch, 0:1],
                            bias=beta_sb[:, cch:cch + 1],
                        )
                        for ti, eng in DW_OFFLOAD:
                            e = nc.gpsimd if eng == "pool" else nc.vector
                            e.scalar_tensor_tensor(
                                out=yv[:], in0=_xw(st, cch, hb, ti).bitcast(F32),
                                scalar=kcol_sb[:, cch, ti:ti + 1],
                                in1=yv[:], op0=ALU.mult, op1=ALU.add,
                            )
                    yp = ps_mm.tile([128, HBLK * W], F32, tag="mm", name="yp")
                    for i, ti in enumerate(pe_taps if off else range(9)):
                        nc.tensor.matmul(
                            yp[:],
                            _r(diag_sb[:, cch, ti, :]),
                            _r(_xw(st, cch, hb, ti)),
                            start=(i == 0),
                            stop=(i == (len(pe_taps) if off else 9) - 1),
                            skip_group_check=True,
                        )
                    if off:
                        nc.vector.scalar_tensor_tensor(
                            out=_r(ysl), in0=yp[:],
                            scalar=1.0,
                            in1=yv.rearrange("p h w -> p (h w)"),
                            op0=ALU.mult, op1=ALU.add)
                    else:
                        nc.scalar.activation(
                            out=_r(ysl),
                            in_=yp[:],
                            func=AF.Identity,
                            bias=beta_sb[:, cch:cch + 1],
                        )

            def v_block(st, nb):
                for vc in range(NCH):
                    vp = ps_mm.tile([128, 512], F32, tag="mm", name="vp")
                    for kc in range(NCH):
                        nc.tensor.matmul(
                            vp[:],
                            _r(pwv_sb[:, kc, vc, :]),
                            _r(st["yt"][:, kc, nb * 512:(nb + 1) * 512]),
                            start=(kc == 0),
                            stop=(kc == NCH - 1),
                        )
                    if VT_EVICT_DVE:
                        nc.vector.tensor_scalar(
                            out=st["vt"][:, vc, nb * 512:(nb + 1) * 512],
                            in0=vp[:], scalar1=vb_sb[:, vc:vc + 1],
                            scalar2=None, op0=ALU.add,
                        )
                    else:
                        nc.scalar.activation(
                            out=st["vt"][:, vc, nb * 512:(nb + 1) * 512],
                            in_=vp[:], func=AF.Identity,
                            bias=vb_sb[:, vc:vc + 1],
                        )

            def qk_front(st, p):
                # two token chunks t0,t1: qkv matmuls -> squares (ACT) ->
                # grouped reduce (GpSimd) -> w = rsqrt(sqq*sqk) -> qs (bf16,
                # q*w straight from PSUM) and k (bf16) evictions. The gram
                # matmuls are emitted later (qk_gram) so other PE work covers
                # this vector-side latency.
                qps = []
                sqs = []
                for j in (0, 1):
                    t = 2 * p + j
                    qp = ps_qk.tile([128, 512], F32, tag="qk", name=f"qp{j}")
                    for kc in range(NCH):
                        nc.tensor.matmul(
                            qp[:],
                            _r(st["yt"][:, kc, t * 128:(t + 1) * 128]),
                            _r(pwqk_sb[:, kc, :]),
                            start=(kc == 0),
                            stop=(kc == NCH - 1),
                        )
                    if add_qbias:
                        nc.vector.tensor_tensor(
                            out=qp[:, 0:256], in0=qp[:, 0:256],
                            in1=qb_sb[:], op=ALU.add,
                        )
                    sq = sq_pool.tile([128, 512], F32, name="sq")
                    nc.scalar.square(sq[:], qp[:])
                    qps.append(qp)
                    sqs.append(sq)
                sqr = wp_pool.tile([128, 2, 16], F32, tag="sqr", name="sqr")
                for j in (0, 1):
                    sqv = sqs[j].rearrange("p (g d) -> p g d", d=HD)
                    if SQ_FOLD_POOL:
                        sqh = sq_pool.tile([128, 16, 16], F32, tag="sqh",
                                           name="sqh")
                        nc.gpsimd.tensor_tensor(
                            out=sqh[:], in0=sqv[:, :, 0:16],
                            in1=sqv[:, :, 16:32], op=ALU.add,
                        )
                        sqv = sqh
                    nc.vector.tensor_reduce(
                        out=sqr[:, j, :],
                        in_=sqv,
                        axis=mybir.AxisListType.X,
                        op=ALU.add,
                    )
                # w = rsqrt(sqq*sqk) = sqrt(1/(sqq*sqk)); s folded into softmax
                w = wp_pool.tile([128, 2, 8], F32, tag="w", name="w")
                wf = w.rearrange("p a h -> p (a h)")
                nc.vector.tensor_tensor(
                    out=w[:], in0=sqr[:, :, 0:8], in1=sqr[:, :, 8:16],
                    op=ALU.mult,
                )
                nc.vector.reciprocal(wf, wf)
                nc.scalar.activation(wf, wf, AF.Sqrt)
                for j in (0, 1):
                    t = 2 * p + j
                    nc.vector.tensor_tensor(
                        out=st["qs_all"][:, t, :].rearrange(
                            "p (h d) -> p h d", d=HD),
                        in0=qps[j][:, 0:256].rearrange("p (h d) -> p h d", d=HD),
                        in1=w[:, j, :].unsqueeze(2).broadcast_to([128, 8, HD]),
                        op=ALU.mult,
                    )
                    if KCOPY_ACT:
                        nc.scalar.copy(st["kb_all"][:, t, :], qps[j][:, 256:512])
                    else:
                        nc.vector.tensor_copy(
                            st["kb_all"][:, t, :], qps[j][:, 256:512])

            def qk_gram(st, p, g=0):
                for j in (0, 1):
                    t = 2 * p + j
                    nc.tensor.matmul(
                        st["att"][:],
                        st["qs_all"][:, t, g * 128:(g + 1) * 128],
                        st["kb_all"][:, t, g * 128:(g + 1) * 128],
                        start=(t == 0),
                        stop=(t == NBLK - 1),
                    )

            def softmax_g(st, g):
                if g == 0:
                    st["at_bd"] = small.tile([128, 2, 128], BF16, tag="atbd",
                                             name="at_bd")
                at_bd = st["at_bd"]
                asm = small.tile([128, 32], F32, tag="asm", name="asm")
                for j in range(4):
                    h = 4 * g + j
                    nc.scalar.mul(
                        asm[32 * j:32 * j + 32, :],
                        st["att"][32 * j:32 * j + 32, 32 * j:32 * j + 32],
                        s_host[h],
                    )
                mx = small.tile([128, 1], F32, tag="mx", name="mx")
                nc.vector.tensor_reduce(
                    out=mx[:], in_=asm[:], axis=mybir.AxisListType.X,
                    op=ALU.max, negate=True)
                nc.scalar.activation(asm[:], asm[:], AF.Exp, bias=mx[:])
                sm = small.tile([128, 1], F32, tag="sm", name="sm")
                nc.vector.tensor_reduce(
                    out=sm[:], in_=asm[:], axis=mybir.AxisListType.X,
                    op=ALU.add)
                nc.vector.reciprocal(sm[:], sm[:])
                nc.vector.tensor_scalar(
                    out=asm[:], in0=asm[:], scalar1=sm[:], scalar2=None,
                    op0=ALU.mult)
                atf = small.tile([128, 128], F32, tag="atf", name="atf")
                nc.vector.memset(atf[:], 0.0)
                for j in range(4):
                    nc.vector.transpose(
                        atf[32 * j:32 * j + 32, 32 * j:32 * j + 32],
                        asm[32 * j:32 * j + 32, :],
                    )
                nc.vector.tensor_copy(at_bd[:, g, :], atf[:])

            def c_block(st, nb, tail=False):
                # one 512-token slab: attn^T @ v then proj + store. In the
                # trailing (non-interleaved) loop the attn@v tiles borrow the
                # idle ps_qk banks for deeper rotation.
                img = st["img"]
                if nb == 0:
                    st["ocm"] = img_pool.tile([128, NCH, N], BF16, tag="ocm",
                                              name=f"ocm{img}")
                ocm = st["ocm"]
                for g in range(NCH):
                    op_ = (ps_qk if tail else ps_c).tile(
                        [128, 512], F32, tag="qk" if tail else "cmm",
                        name="op_")
                    nc.tensor.matmul(
                        op_[:],
                        st["at_bd"][:, g, :],
                        st["vt"][:, g, nb * 512:(nb + 1) * 512],
                    )
                    if g == 0:
                        nc.vector.tensor_copy(
                            ocm[:, g, nb * 512:(nb + 1) * 512], op_[:])
                    else:
                        nc.scalar.copy(
                            ocm[:, g, nb * 512:(nb + 1) * 512], op_[:])
                for t in range(4 * nb, 4 * nb + 4):
                    pp = ps_c.tile([128, 256], F32, tag="cmm", name="pp")
                    for kc in range(NCH):
                        nc.tensor.matmul(
                            pp[:],
                            ocm[:, kc, t * 128:(t + 1) * 128],
                            projw_sb[:, kc, :],
                            start=(kc == 0),
                            stop=(kc == NCH - 1),
                        )
                    if t % 2 == 0:
                        ot = ostage.tile([128, 2, 256], F32, name="ot")
                        st["ot"] = ot
                    ot = st["ot"]
                    if add_pbias:
                        nc.vector.tensor_tensor(
                            out=ot[:, t % 2, :], in0=pp[:], in1=pb_sb[:],
                            op=ALU.add)
                    elif t % 2 == 0:
                        nc.scalar.copy(ot[:, t % 2, :], pp[:])
                    else:
                        nc.vector.tensor_copy(ot[:, t % 2, :], pp[:])
                    if t % 2 == 1:
                        nc.sync.dma_start(
                            out_dr[img, (t - 1) * 128:(t + 1) * 128,
                                   :].rearrange("(g p) c -> p g c", p=128),
                            ot[:],
                        )

            def phase_A(st, interleave=None):
                # interleave: optional callable(tb) emitting prev-img C blocks
                def qk_v_body(hb):
                    # consumes yt of hb (evicted a full stage earlier, so PE
                    # never waits on the eviction). qk_gram(p) is deferred
                    # past qk_front(p+1) + a v_block so PE never stalls on
                    # the pair's vector-side chain either.
                    qk_front(st, 2 * hb)
                    qk_front(st, 2 * hb + 1)
                    if hb >= 2:
                        v_block(st, hb - 2)
                    qk_gram(st, 2 * hb)
                    qk_gram(st, 2 * hb + 1)

                for s in range(8):
                    if s == 0:
                        load_rb(st, 0)
                        load_rb(st, 1)
                    elif s < 7:
                        load_rb(st, s + 1)
                    if interleave is not None:
                        interleave(s)
                    dwconv_block(st, s)
                    if s >= 1:
                        qk_v_body(s - 1)
                qk_v_body(7)
                v_block(st, 6)
                v_block(st, 7)
                # head-group 0 gram finished accumulating; softmax it, then
                # re-run the gram for group 1 from the stored qs/kb (the att
                # bank is reused, so g1's start must come after g0 is read).
                softmax_g(st, 0)
                for p in range(NBLK // 2):
                    qk_gram(st, p, g=1)
                softmax_g(st, 1)

            import contextlib
            rep_engines = (mybir.EngineType.PE, mybir.EngineType.DVE,
                           mybir.EngineType.Activation, mybir.EngineType.SP,
                           mybir.EngineType.Pool)
            rep_ctx = (tc.For_i(0, reps, 1, hint_engines=rep_engines)
                       if reps > 1 else contextlib.nullcontext())
            with rep_ctx:
                prev = None
                for img in range(IMGS):
                    st = make_img_state(img)
                    if prev is None:
                        phase_A(st)
                    else:
                        pv = prev

                        def emit_c(tb, pv=pv):
                            if tb < 8:
                                c_block(pv, tb)
                        phase_A(st, interleave=emit_c)
                    prev = st
                for nb in range(8):
                    c_block(prev, nb, tail=True)

    nc.finalize()
    return nc


def _prep_consts(dw_kernel, bn_gamma, bn_beta, pw_kernel, q_bias, v_bias,
                 scale, proj_w, proj_b):
    taps_w = np.empty((9, C), np.float32)
    for ti, (dh, dw) in enumerate(TAPS):
        taps_w[ti] = dw_kernel[dh + 1, dw + 1, 0, :] * bn_gamma

    diag = np.zeros((128, NCH, 9, 128), np.float32)
    idx = np.arange(128)
    for cch in range(NCH):
        for ti in range(9):
            diag[idx, cch, ti, idx] = taps_w[ti, cch * 128 + idx]

    pwqk = np.empty((128, NCH, 512), np.float32)
    pwv = np.empty((128, NCH, NCH, 128), np.float32)
    for kc in range(NCH):
        pwqk[:, kc, :] = pw_kernel[kc * 128:(kc + 1) * 128, 0:512]
        for vc in range(NCH):
            pwv[:, kc, vc, :] = pw_kernel[kc * 128:(kc + 1) * 128,
                                          512 + vc * 128:512 + (vc + 1) * 128]

    projw = np.empty((128, NCH, 256), ml_dtypes.bfloat16)
    for kc in range(NCH):
        projw[:, kc, :] = proj_w[kc * 128:(kc + 1) * 128, :].astype(ml_dtypes.bfloat16)

    s_host = np.exp(np.minimum(scale.reshape(HEADS), LOG_MAX_SCALE)).astype(np.float32)

    # tap weights as per-partition columns [128, NCH, 9] for vector-engine
    # dwconv tap offload (scalar_tensor_tensor / activation scale operands)
    kcol = np.empty((128, NCH, 9), np.float32)
    for cch in range(NCH):
        for ti in range(9):
            kcol[:, cch, ti] = taps_w[ti, cch * 128:(cch + 1) * 128]

    consts = {
        "diag": diag,
        "pwqk": pwqk,
        "pwv": pwv,
        "projw": projw,
        "beta": bn_beta.reshape(NCH, 128).T.astype(np.float32).copy(),
        "vb": v_bias.reshape(NCH, 128).T.astype(np.float32).copy(),
        "kcol": kcol,
        "s_host": s_host,
        "qb": np.tile(q_bias[None, :], (128, 1)).astype(np.float32),
        "pb": np.tile(proj_b[None, :], (128, 1)).astype(np.float32),
    }
    return consts


def make_in_maps(x):
    # per-core channel-major x: [IMGS, C, N]
    xs = np.ascontiguousarray(
        np.asarray(x, np.float32).reshape(NCORES, IMGS, N, C).transpose(
            0, 1, 3, 2))
    return [{"x": xs[i]} for i in range(NCORES)]


def kernel(x, dw_kernel, bn_gamma, bn_beta, pw_kernel, q_bias, v_bias, scale,
           proj_w, proj_b):
    consts = _prep_consts(
        np.asarray(dw_kernel, np.float32), np.asarray(bn_gamma, np.float32),
        np.asarray(bn_beta, np.float32), np.asarray(pw_kernel, np.float32),
        np.asarray(q_bias, np.float32), np.asarray(v_bias, np.float32),
        np.asarray(scale, np.float32), np.asarray(proj_w, np.float32),
        np.asarray(proj_b, np.float32))

    add_qbias = bool(np.any(q_bias))
    add_pbias = bool(np.any(proj_b))
    nc = _build_program(consts, add_qbias, add_pbias)

    in_maps = make_in_maps(x)
    res = run_bass_kernel_spmd(nc, in_maps, core_ids=list(range(NCORES)))
    out = np.stack([res.results[i]["out"] for i in range(NCORES)])
    return out.reshape(B, H, W, C)


if __name__ == "__main__":
    pass
